# revision 1
# baseline (speedup 1.0000x reference)
"""Trainium2 Bass kernel for nn_CONV_A_64115271795341.

The module (im2col mean-centered conv + linear on window means) folds exactly
into a single 3x3 edge-padded convolution with effective weights:

  W_eff[c,k,d] = weight[c,k,d] + (w_lin[d,c] - sum_k weight[c,k,d]) / 9

Sharding: data-parallel over batch (8 images -> 8 NeuronCores), weights
replicated.

Per-core design:
  - host pre-pads each image to [64, 130*130] (edge padding), shipped fp32r.
  - SBUF xp[128, NP] fp32r: partitions 0-63 = padded image, partitions
    64-127 = same data shifted +1 element (DVE copy). A K=128 matmul at
    base offset o contracts taps j and j+1 at once: "pair" matmuls cover
    kernel taps (i,0)+(i,1) for each kernel row i.
  - taps (i,2) run as bf16 K=64 matmuls on PE column-groups 2-3
    (tile_position=(0,64)) reading a bf16 copy made by GPSIMD; fp32r
    matmuls cannot write dst partitions 64-127, bf16 can, and this keeps
    both halves of the PE array busy concurrently.
  - epilogue fused over pairs of output tiles: ACT copies PSUM-B to SBUF,
    DVE adds PSUM-A + SBUF, HWDGE stores [64, 1024] chunks.
"""

import numpy as np

C, H, W, D, B = 64, 128, 128, 64, 8
KS = 3
WP = W + 2            # 130
HP = H + 2
NP = WP * HP          # 16900 padded elems
TILE_ROWS = 4
NTILES = H // TILE_ROWS          # 32 tiles of [64, 512]
TN = TILE_ROWS * W               # 512
GROUP = 2                        # output tiles fused per epilogue op
NGROUPS = NTILES // GROUP
IN_CHUNKS = 8
DUP_CHUNKS = 4
CAST_CHUNKS = 8

_CACHE = {}


def _build(repeat=1, in_chunks=IN_CHUNKS, dup_chunks=DUP_CHUNKS,
           cast_chunks=CAST_CHUNKS, singles=True, group=GROUP,
           psum_bufs=2, out_engine="scalar", pairs=True,
           cast_engine="gpsimd", epilogue="auto", xb_host=False,
           xb_dtype="float16", allfp16=False, seq_singles=False,
           out_fp16=False):
    import concourse.bass as bass  # noqa: F401
    import concourse.mybir as mybir
    import concourse.tile as tile
    from concourse import bacc

    dt = mybir.dt
    nc = bacc.Bacc("TRN2", target_bir_lowering=False, debug=False, num_devices=8)

    if allfp16:
        return _build_allfp16(nc, mybir, tile, repeat=repeat, in_chunks=in_chunks,
                              dup_chunks=dup_chunks, group=group,
                              psum_bufs=psum_bufs, seq_singles=seq_singles,
                              out_fp16=out_fp16)
    xdt = getattr(dt, xb_dtype)
    x_d = nc.dram_tensor("x", [C, NP], dt.float32r, kind="ExternalInput")
    wpair_d = nc.dram_tensor("wpair", [128, 3 * D], dt.float32r, kind="ExternalInput")
    wsing_d = nc.dram_tensor("wsing", [C, 3 * D], xdt, kind="ExternalInput")
    if xb_host:
        xb_d = nc.dram_tensor("xb", [C, NP], xdt, kind="ExternalInput")
    out_d = nc.dram_tensor("out", [D, H * W], dt.float32, kind="ExternalOutput")

    with tile.TileContext(nc) as tc:
        with tc.tile_pool(name="io", bufs=1) as io_pool, \
             tc.tile_pool(name="outp", bufs=3) as out_pool, \
             tc.tile_pool(name="tmpp", bufs=2) as tmp_pool, \
             tc.tile_pool(name="psa", bufs=psum_bufs, space="PSUM") as psa_pool, \
             tc.tile_pool(name="psb", bufs=psum_bufs, space="PSUM") as psb_pool:

            for _rep in range(repeat):
                wpair_sb = io_pool.tile([128, 3 * D], dt.float32r,
                                        name="wpair_sb", tag="wpair_sb")
                nc.sync.dma_start(wpair_sb[:, :], wpair_d.ap()[:, :])
                wsing_sb = io_pool.tile([C, 3 * D], xdt,
                                        name="wsing_sb", tag="wsing_sb")
                nc.sync.dma_start(wsing_sb[:, :], wsing_d.ap()[:, :])

                xp = io_pool.tile([128, NP], dt.float32r, name="xp", tag="xp")
                xb = io_pool.tile([C, NP], xdt, name="xb", tag="xb")

                bnd = [NP * g // max(in_chunks, 1) for g in range(in_chunks + 1)]
                for g in range(in_chunks):
                    a, b = bnd[g], bnd[g + 1]
                    nc.sync.dma_start(xp[0:C, a:b], x_d.ap()[:, a:b])
                dbnd = [NP * g // max(dup_chunks, 1) for g in range(dup_chunks + 1)]
                for g in range(dup_chunks):
                    a, b = dbnd[g], dbnd[g + 1]
                    be = min(b, NP - 1)
                    nc.vector.tensor_copy(xp[C:128, a:be], xp[0:C, a + 1:be + 1])
                if xb_host:
                    xbnd = [NP * g // max(in_chunks, 1) for g in range(in_chunks + 1)]
                    for g in range(in_chunks):
                        a, b = xbnd[g], xbnd[g + 1]
                        nc.sync.dma_start(xb[:, a:b], xb_d.ap()[:, a:b])
                else:
                    cbnd = [NP * g // max(cast_chunks, 1) for g in range(cast_chunks + 1)]
                    cast_eng = {"gpsimd": nc.gpsimd, "vector": nc.vector,
                                "scalar": nc.scalar}[cast_engine]
                    for g in range(cast_chunks):
                        a, b = cbnd[g], cbnd[g + 1]
                        if cast_engine == "scalar":
                            nc.scalar.copy(xb[:, a:b], xp[0:C, a:b].bitcast(dt.float32))
                        else:
                            cast_eng.tensor_copy(xb[:, a:b], xp[0:C, a:b].bitcast(dt.float32))

                xv = xp.rearrange("p (r c) -> p r c", c=WP)
                xbv = xb.rearrange("p (r c) -> p r c", c=WP)

                ngroups = NTILES // group
                for grp in range(ngroups):
                    if not pairs and not singles:
                        # DMA-path-only variants: ship arbitrary bytes out
                        nc.scalar.dma_start(
                            out_d.ap()[:, group * TN * grp:group * TN * (grp + 1)],
                            xp[0:C, group * TN * grp:group * TN * (grp + 1)].bitcast(mybir.dt.float32))
                        continue
                    psA = (psa_pool.tile([64, group * TN], mybir.dt.float32,
                                         name="psA", tag="psA") if pairs else None)
                    psB = (psb_pool.tile([128, group * TN], mybir.dt.float32,
                                         name="psB", tag="psB") if singles else None)
                    for s in range(group):
                        t = grp * group + s
                        h0 = t * TILE_ROWS
                        for i in range(KS):
                            if pairs:
                                nc.tensor.matmul(
                                    psA[:, TN * s:TN * (s + 1)],
                                    lhsT=wpair_sb[:, D * i:D * (i + 1)],
                                    rhs=xv[:, h0 + i:h0 + i + TILE_ROWS, 0:W],
                                    start=(i == 0), stop=(i == KS - 1),
                                )
                            if singles:
                                nc.tensor.matmul(
                                    psB[64:128, TN * s:TN * (s + 1)],
                                    lhsT=wsing_sb[:, D * i:D * (i + 1)],
                                    rhs=xbv[:, h0 + i:h0 + i + TILE_ROWS, 2:WP],
                                    start=(i == 0), stop=(i == KS - 1),
                                    tile_position=(0, 64),
                                )
                    outt = out_pool.tile([64, group * TN], mybir.dt.float32,
                                         name="outt", tag="outt")
                    epi = epilogue
                    if epi == "auto":
                        epi = "add" if (singles and pairs) else ("copyB" if singles else "copyA")
                    if epi == "add":
                        tmp = tmp_pool.tile([64, group * TN], mybir.dt.float32,
                                            name="tmp", tag="tmp")
                        nc.scalar.copy(tmp[:, :], psB[64:128, :])
                        nc.vector.tensor_add(outt[:, :], psA[:, :], tmp[:, :])
                    elif epi == "copyB":
                        nc.vector.tensor_copy(outt[:, :], psB[64:128, :])
                    else:
                        nc.vector.tensor_copy(outt[:, :], psA[:, :])
                    dma_eng = nc.scalar if out_engine == "scalar" else nc.sync
                    dma_eng.dma_start(
                        out_d.ap()[:, group * TN * grp:group * TN * (grp + 1)],
                        outt[:, :])

    nc.compile()
    return nc


def _build_allfp16(nc, mybir, tile, repeat=1, in_chunks=8, dup_chunks=4,
                   group=GROUP, psum_bufs=2, seq_singles=False,
                   out_fp16=False):
    """All-fp16 datapath.

    Ships only the fp16 padded image (2.1MB in). Pairs (K=128, taps
    (i,0)+(i,1) via [top; top-shifted-by-1]) run on PE column-groups 0-1
    into psA[0:64]; singles (K=64, taps (i,2), top half) run concurrently
    on column-groups 2-3 into psB[64:128]. psA/psB are separate tensors so
    their banks are disjoint (no BankOverlapTracker serialization).
    Epilogue per 2-tile group: ACT copies psB->SBUF, DVE adds psA+tmp,
    one HWDGE store.
    """
    dt = mybir.dt
    xb_d = nc.dram_tensor("xb", [C, NP], dt.float16, kind="ExternalInput")
    wpair_d = nc.dram_tensor("wpair16", [128, 3 * D], dt.float16, kind="ExternalInput")
    wsing_d = nc.dram_tensor("wsing", [C, 3 * D], dt.float16, kind="ExternalInput")
    odt = dt.float16 if out_fp16 else dt.float32
    out_d = nc.dram_tensor("out", [D, H * W], odt, kind="ExternalOutput")

    sing_col = 0 if seq_singles else 64

    with tile.TileContext(nc) as tc:
        with tc.tile_pool(name="io", bufs=1) as io_pool, \
             tc.tile_pool(name="outp", bufs=3) as out_pool, \
             tc.tile_pool(name="tmpp", bufs=2) as tmp_pool, \
             tc.tile_pool(name="psa", bufs=psum_bufs, space="PSUM") as psa_pool, \
             tc.tile_pool(name="psb", bufs=psum_bufs, space="PSUM") as psb_pool:

            for _rep in range(repeat):
                wpair_sb = io_pool.tile([128, 3 * D], dt.float16,
                                        name="wpair_sb", tag="wpair_sb")
                nc.sync.dma_start(wpair_sb[:, :], wpair_d.ap()[:, :])
                wsing_sb = io_pool.tile([C, 3 * D], dt.float16,
                                        name="wsing_sb", tag="wsing_sb")
                nc.sync.dma_start(wsing_sb[:, :], wsing_d.ap()[:, :])

                xp = io_pool.tile([128, NP], dt.float16, name="xp", tag="xp")

                bnd = [NP * g // in_chunks for g in range(in_chunks + 1)]
                for g in range(in_chunks):
                    a, b = bnd[g], bnd[g + 1]
                    nc.sync.dma_start(xp[0:C, a:b], xb_d.ap()[:, a:b])
                dbnd = [NP * g // dup_chunks for g in range(dup_chunks + 1)]
                for g in range(dup_chunks):
                    a, b = dbnd[g], dbnd[g + 1]
                    be = min(b, NP - 1)
                    nc.vector.tensor_copy(xp[C:128, a:be], xp[0:C, a + 1:be + 1])

                xv = xp.rearrange("p (r c) -> p r c", c=WP)

                ngroups = NTILES // group
                for grp in range(ngroups):
                    psA = psa_pool.tile([64, group * TN], mybir.dt.float32,
                                        name="psA", tag="psA")
                    psB = psb_pool.tile([128, group * TN], mybir.dt.float32,
                                        name="psB", tag="psB")
                    for s in range(group):
                        t = grp * group + s
                        h0 = t * TILE_ROWS
                        for i in range(KS):
                            nc.tensor.matmul(
                                psA[:, TN * s:TN * (s + 1)],
                                lhsT=wpair_sb[:, D * i:D * (i + 1)],
                                rhs=xv[:, h0 + i:h0 + i + TILE_ROWS, 0:W],
                                start=(i == 0), stop=(i == KS - 1),
                            )
                            nc.tensor.matmul(
                                psB[sing_col:sing_col + 64, TN * s:TN * (s + 1)],
                                lhsT=wsing_sb[:, D * i:D * (i + 1)],
                                rhs=xv[0:C, h0 + i:h0 + i + TILE_ROWS, 2:WP],
                                start=(i == 0), stop=(i == KS - 1),
                                tile_position=(0, sing_col),
                            )
                    tmp = tmp_pool.tile([64, group * TN], mybir.dt.float32,
                                        name="tmp", tag="tmp")
                    nc.scalar.copy(tmp[:, :], psB[sing_col:sing_col + 64, :])
                    outt = out_pool.tile([64, group * TN], odt,
                                         name="outt", tag="outt")
                    nc.vector.tensor_add(outt[:, :], psA[:, :], tmp[:, :])
                    nc.scalar.dma_start(
                        out_d.ap()[:, group * TN * grp:group * TN * (grp + 1)],
                        outt[:, :])

    nc.compile()
    return nc


def _prep_inputs(x, weight, w_lin):
    import ml_dtypes
    w = weight.astype(np.float64)
    weff = w + (w_lin.astype(np.float64).T[:, None, :] - w.sum(axis=1, keepdims=True)) / 9.0
    weff = weff.astype(np.float32)                      # [C, 9, D]
    wpair = np.empty((128, 3 * D), np.float32)
    wsing = np.empty((C, 3 * D), np.float32)
    for i in range(KS):
        wpair[0:C, D * i:D * (i + 1)] = weff[:, 3 * i + 0, :]
        wpair[C:128, D * i:D * (i + 1)] = weff[:, 3 * i + 1, :]
        wsing[:, D * i:D * (i + 1)] = weff[:, 3 * i + 2, :]
    wsing = wsing.astype(np.float16)
    wpair16 = wpair.astype(np.float16)

    xp = np.pad(np.asarray(x), ((0, 0), (0, 0), (1, 1), (1, 1)), mode="edge")
    xp = xp.reshape(B, C, NP).astype(np.float32)
    xb = xp.astype(np.float16)
    return xp, wpair, wsing, xb, wpair16


OUT_FP16 = False


def kernel(x, weight, w_lin):
    from concourse.bass_utils import run_bass_kernel_spmd

    if "nc" not in _CACHE:
        _CACHE["nc"] = _build(allfp16=True, out_fp16=OUT_FP16)
    nc = _CACHE["nc"]

    xp, wpair, wsing, xb, wpair16 = _prep_inputs(x, weight, w_lin)
    in_maps = [
        {"xb": xb[b], "wpair16": wpair16, "wsing": wsing}
        for b in range(B)
    ]
    res = run_bass_kernel_spmd(nc, in_maps, core_ids=list(range(B)))
    out = np.stack([res.results[b]["out"].reshape(D, H, W) for b in range(B)])
    return out.astype(np.float32)

